# revision 26
# baseline (speedup 1.0000x reference)
"""ColBERT MaxSim retrieval kernel for 8 Trainium2 NeuronCores.

Problem (per reference):
  Q  = l2norm(q_hidden @ W + b)                    [B, 32, 128]
  PD = l2norm((pd_hidden @ W + b) * pd_mask)       [B, 512, 128]
  ND = l2norm((nd_hidden @ W + b) * nd_mask)       [B, 512, 128]
  pos = einsum(Q, PD).max(k).sum(q);  neg likewise; out = [B, 2]

Sharding: pure data parallelism - batch dim (128) split across 8 cores
(16 batches each); W, b replicated.

Key layout decision: the PE contracts along the partition dim, so doc
hidden states must be fed as X^T [H-part, L-free]. Rather than burn PE
cycles transposing on-chip (the v0 bottleneck: 792 PE transposes +
PSUM->SBUF copies), the host pre-transposes each 512-token tile into
[128, 6, 512] bf16 blocks laid out so every per-partition DMA read is
one contiguous 6KB stream.

Normalization trick: never materialize normalized PD. With
  S_raw[q,k] = (Qn @ (Xd W + b)^T)[q,k]
  cs[k] = mask[k] * rsqrt(||Xd_k W + b||^2)
scores are S_raw * cs (masked columns exactly 0, matching the reference
where masked tokens are zero vectors), so pos = sum_q max_k (S_raw*cs).

Per-tile PE work is just: 6-chain projection matmul, one row-sum matmul
(norms, packed 4 tiles/psum-tile via tile_position), one MaxSim matmul.
Per group of 4 tiles: a single K=4 broadcast matmul expands the 4 norm
rows to the 128 score partitions.
"""

import os
import sys

import numpy as np

for _p in ("/opt/trn_rl_repo",):
    if _p not in sys.path and os.path.isdir(_p):
        sys.path.insert(0, _p)

import ml_dtypes  # noqa: E402

import concourse.bass as bass  # noqa: E402
import concourse.bacc as bacc  # noqa: E402
import concourse.tile as tile  # noqa: E402
from concourse import mybir  # noqa: E402
from concourse.bass_utils import run_bass_kernel_spmd  # noqa: E402

# Problem shape (hardcoded per contract)
B, LQ, LD, H, D = 128, 32, 512, 768, 128
NCORES = 8
BC = B // NCORES          # 16 batches per core
KT = H // 128             # 6 contraction chunks

F32 = mybir.dt.float32
BF16 = mybir.dt.bfloat16
AF = mybir.ActivationFunctionType
ALU = mybir.AluOpType

BF16_NP = ml_dtypes.bfloat16


def build_kernel():
    nc = bacc.Bacc()

    # Pre-transposed per-tile layouts: [tile, 128 (h%128), KT*512 (h//128, l)]
    qt_d = nc.dram_tensor("qt", [128, KT * LD], BF16, kind="ExternalInput")
    pdt_d = nc.dram_tensor("pdt", [BC, 128, KT * LD], BF16, kind="ExternalInput")
    ndt_d = nc.dram_tensor("ndt", [BC, 128, KT * LD], BF16, kind="ExternalInput")
    w_d = nc.dram_tensor("W", [128, KT, 128], BF16, kind="ExternalInput")
    b_d = nc.dram_tensor("b", [D, 1], F32, kind="ExternalInput")
    # masks as [j, u, l]: BIG * (1 - mask[4u+j, l]) — added to squared norms
    # so masked tokens get inverse-norm ~1e-9 (scores ~1e-18, below tol)
    mp_d = nc.dram_tensor("mp", [4, 4, LD], BF16, kind="ExternalInput")
    mn_d = nc.dram_tensor("mn", [4, 4, LD], BF16, kind="ExternalInput")
    i4_d = nc.dram_tensor("i4", [4, 4], BF16, kind="ExternalInput")
    blk4_d = nc.dram_tensor("blk4", [4, 128], BF16, kind="ExternalInput")
    e4_d = nc.dram_tensor("e4", [128, 4], BF16, kind="ExternalInput")
    # ej4[p, j, c] = (c == j): routes a full-column reduction to out row j
    ej4_d = nc.dram_tensor("ej4", [128, 4, 4], BF16, kind="ExternalInput")
    out_d = nc.dram_tensor("out", [BC, 2], F32, kind="ExternalOutput")

    with tile.TileContext(nc) as tc:
        with (
            tc.tile_pool(name="const", bufs=1) as const,
            tc.tile_pool(name="xin", bufs=6) as xin,
            tc.tile_pool(name="ptb", bufs=4) as ptbp,
            tc.tile_pool(name="sq", bufs=4) as sqp,
            tc.tile_pool(name="small", bufs=4) as smallp,
            tc.tile_pool(name="csr", bufs=2) as csrp,
            tc.tile_pool(name="persist", bufs=1) as persist,
            tc.tile_pool(name="ptps", bufs=3, space="PSUM") as ptpsp,
            tc.tile_pool(name="ssps", bufs=2, space="PSUM") as sspsp,
            tc.tile_pool(name="s4ps", bufs=2, space="PSUM") as s4psp,
            tc.tile_pool(name="bcps", bufs=1, space="PSUM") as bcpsp,
        ):
            # ---- constants ----
            w_sb = const.tile([128, KT, 128], BF16)
            nc.sync.dma_start(out=w_sb, in_=w_d[:, :, :])
            bias_sb = const.tile([128, 1], F32)
            nc.sync.dma_start(out=bias_sb, in_=b_d[:, :])
            mp_sb = const.tile([4, 4, LD], BF16)
            nc.sync.dma_start(out=mp_sb, in_=mp_d[:, :, :])
            mn_sb = const.tile([4, 4, LD], BF16)
            nc.sync.dma_start(out=mn_sb, in_=mn_d[:, :, :])
            blk4_sb = const.tile([4, 128], BF16)
            nc.sync.dma_start(out=blk4_sb, in_=blk4_d[:, :])
            e4_sb = const.tile([128, 4], BF16)
            nc.sync.dma_start(out=e4_sb, in_=e4_d[:, :])
            ej4_sb = const.tile([128, 4, 4], BF16)
            nc.sync.dma_start(out=ej4_sb, in_=ej4_d[:, :, :])
            i4_sb = const.tile([4, 4], BF16)
            nc.sync.dma_start(out=i4_sb, in_=i4_d[:, :])
            ones_col = const.tile([128, 1], BF16)
            nc.vector.memset(ones_col, 1.0)
            ones_row = const.tile([1, 128], BF16)
            nc.vector.memset(ones_row, 1.0)

            # warm the scalar activation tables while DMAs run
            warm_sb = const.tile([1, 2], BF16)
            nc.scalar.activation(warm_sb, ones_row[0:1, 0:2], AF.Square)
            nc.scalar.activation(warm_sb, ones_row[0:1, 0:2], AF.Abs_reciprocal_sqrt)

            rm_sb = persist.tile([128, 8], BF16)
            qtn_sb = persist.tile([128, BC * LQ], BF16)

            def project(xt_sb):
                """6-chain matmul: xt [128, KT, 512] -> P^T psum [128, 512]."""
                pt_ps = ptpsp.tile([128, LD], F32, tag="pt")
                for k in range(KT):
                    nc.tensor.matmul(
                        pt_ps,
                        w_sb[:, k, :],
                        xt_sb[:, k, :],
                        start=(k == 0),
                        stop=(k == KT - 1),
                    )
                return pt_ps

            # ---- prefetch first two doc tiles on both queues, then query ----
            preloaded = {}
            for b0, eng in ((0, nc.sync), (1, nc.gpsimd)):
                xt_sb = xin.tile([128, KT, LD], BF16, tag="x")
                eng.dma_start(
                    out=xt_sb,
                    in_=pdt_d[b0, :, :].rearrange("p (k l) -> p k l", k=KT),
                )
                preloaded[b0] = xt_sb

            # ---- query stage: all 16 batches (512 query tokens) at once ----
            qx_sb = xin.tile([128, KT, LD], BF16, tag="x")
            nc.gpsimd.dma_start(
                out=qx_sb, in_=qt_d[:, :].rearrange("p (k l) -> p k l", k=KT)
            )
            qpt_ps = project(qx_sb)
            qsq_sb = sqp.tile([128, LD], BF16, tag="sq")
            nc.scalar.activation(qsq_sb, qpt_ps, AF.Square, bias=bias_sb)
            qss_ps = sspsp.tile([4, LD], F32, tag="ss")
            nc.tensor.matmul(
                qss_ps[0:1, :], ones_col, qsq_sb, start=True, stop=True
            )
            qinv_sb = smallp.tile([1, LD], BF16, tag="inv")
            nc.scalar.activation(qinv_sb, qss_ps[0:1, :], AF.Abs_reciprocal_sqrt)
            qbc_ps = bcpsp.tile([128, LD], F32, tag="bc")
            nc.tensor.matmul(qbc_ps, ones_row, qinv_sb, start=True, stop=True)
            qtb_sb = ptbp.tile([128, LD], BF16, tag="ptb")
            nc.vector.tensor_scalar_add(qtb_sb, qpt_ps, bias_sb)
            nc.vector.tensor_mul(qtn_sb, qtb_sb, qbc_ps)

            # ---- doc loop: 4 groups x {pd, nd} x 4 tiles ----
            # Group post-processing is deferred ~2 tiles into the next group
            # so its scalar/vector work doesn't contend with the next tiles'
            # bias-add/square at the group seam.
            def emit_post(u, ti, ss_ps, s4_ps, split=False):
                c = 2 * u + ti
                csrm_sb = csrp.tile([4, LD], BF16, tag="csrm")
                nc.scalar.activation(csrm_sb, ss_ps, AF.Abs_reciprocal_sqrt)
                cs_ps = bcpsp.tile([128, LD], F32, tag="bc")
                csb_sb = ptbp.tile([128, LD], BF16, tag="csb")
                scr_sb = sqp.tile([128, LD], BF16, tag="scr")
                halves = 2 if split else 1
                for h in range(halves):
                    pr = slice(128 // halves * h, 128 // halves * (h + 1))
                    rr = slice(4 // halves * h, 4 // halves * (h + 1))
                    nc.tensor.matmul(
                        cs_ps[pr, :],
                        blk4_sb[rr, pr],
                        csrm_sb[rr, :],
                        start=True,
                        stop=True,
                        tile_position=(0, 128 // halves * h),
                    )
                    nc.scalar.copy(csb_sb[pr, :], cs_ps[pr, :])
                    nc.vector.tensor_mul(scr_sb[pr, :], s4_ps[pr, :], csb_sb[pr, :])
                    nc.vector.tensor_reduce(
                        rm_sb[pr, c : c + 1],
                        scr_sb[pr, :],
                        axis=mybir.AxisListType.X,
                        op=ALU.max,
                    )

            groups = [
                (u, ti, xdram, m_sb)
                for u in range(4)
                for ti, (xdram, m_sb) in enumerate(((pdt_d, mp_sb), (ndt_d, mn_sb)))
            ]
            pending = None
            for u, ti, xdram, m_sb in groups:
                ss_ps = sspsp.tile([4, LD], F32, tag="ss")
                s4_ps = s4psp.tile([128, LD], F32, tag="s4")
                for j in range(4):
                    b = 4 * u + j
                    if ti == 0 and b in preloaded:
                        xt_sb = preloaded.pop(b)
                    else:
                        xt_sb = xin.tile([128, KT, LD], BF16, tag="x")
                        nc.gpsimd.dma_start(
                            out=xt_sb,
                            in_=xdram[b, :, :].rearrange("p (k l) -> p k l", k=KT),
                        )
                    pt_ps = project(xt_sb)
                    ptb_sb = ptbp.tile([128, LD], BF16, tag="ptb")
                    nc.vector.tensor_scalar_add(ptb_sb, pt_ps, bias_sb)
                    sq_sb = sqp.tile([128, LD], BF16, tag="sq")
                    nc.scalar.activation(sq_sb, pt_ps, AF.Square, bias=bias_sb)
                    nc.tensor.matmul(
                        ss_ps,
                        ej4_sb[:, j, :],
                        sq_sb,
                        start=(j == 0),
                        stop=False,
                    )
                    nc.tensor.matmul(
                        s4_ps[32 * j : 32 * (j + 1), :],
                        qtn_sb[:, b * LQ : (b + 1) * LQ],
                        ptb_sb,
                        start=True,
                        stop=True,
                        tile_position=(0, 32 * j),
                    )
                    if pending is not None and j == 1:
                        emit_post(*pending)
                        pending = None
                # close the ss chain: +BIG on masked columns
                nc.tensor.matmul(
                    ss_ps, i4_sb, m_sb[:, u, :], start=False, stop=True
                )
                pending = (u, ti, ss_ps, s4_ps)
            emit_post(*pending)

            # ---- final reduction over queries + output ----
            o44_ps = bcpsp.tile([4, 8], F32, tag="bc")
            nc.tensor.matmul(o44_ps, e4_sb, rm_sb, start=True, stop=True)
            o44_sb = smallp.tile([4, 8], F32, tag="o44sb")
            nc.scalar.copy(o44_sb, o44_ps)
            nc.sync.dma_start(
                out=out_d[:, :].rearrange("(u g) t -> g u t", g=4),
                in_=o44_sb.rearrange("g (u t) -> g u t", t=2),
            )

    nc.compile()
    return nc


_NC_CACHE = None


def _get_nc():
    global _NC_CACHE
    if _NC_CACHE is None:
        _NC_CACHE = build_kernel()
    return _NC_CACHE


def _tileize(x):
    """[rows, H] fp32 -> [rows//512, 128, KT*512] bf16, pre-transposed."""
    nt = x.shape[0] // LD
    xb = x.astype(BF16_NP).reshape(nt, LD, KT, 128).transpose(0, 3, 2, 1)
    return np.ascontiguousarray(xb).reshape(nt, 128, KT * LD)


def _in_maps(inputs):
    q = np.asarray(inputs["q_hidden"], dtype=np.float32)
    pd = np.asarray(inputs["pd_hidden"], dtype=np.float32)
    nd = np.asarray(inputs["nd_hidden"], dtype=np.float32)
    W = np.asarray(inputs["W"], dtype=np.float32)
    b = np.ascontiguousarray(
        np.asarray(inputs["b"], dtype=np.float32).reshape(D, 1)
    )
    w_t = np.ascontiguousarray(
        W.astype(BF16_NP).reshape(KT, 128, D).transpose(1, 0, 2)
    )
    # masks [B, LD] -> per-core [4(j), 4(u), LD] bf16 = BIG * (1 - mask)
    MASK_BIG = 1.0e18
    mp = (
        (1.0 - np.asarray(inputs["pd_mask"], dtype=np.float32)) * MASK_BIG
    ).astype(BF16_NP)
    mn = (
        (1.0 - np.asarray(inputs["nd_mask"], dtype=np.float32)) * MASK_BIG
    ).astype(BF16_NP)
    i4 = np.eye(4, dtype=BF16_NP)
    blk4 = np.zeros((4, 128), dtype=BF16_NP)
    for j in range(4):
        blk4[j, 32 * j : 32 * (j + 1)] = 1
    e4 = np.zeros((128, 4), dtype=BF16_NP)
    for g in range(4):
        e4[32 * g : 32 * (g + 1), g] = 1
    ej4 = np.zeros((128, 4, 4), dtype=BF16_NP)
    for j in range(4):
        ej4[:, j, j] = 1
    maps = []
    for c in range(NCORES):
        sl = slice(c * BC, (c + 1) * BC)
        maps.append(
            {
                "qt": _tileize(q[sl].reshape(BC * LQ, H)).reshape(128, KT * LD),
                "pdt": _tileize(pd[sl].reshape(BC * LD, H)),
                "ndt": _tileize(nd[sl].reshape(BC * LD, H)),
                "W": w_t,
                "b": b,
                "mp": np.ascontiguousarray(
                    mp[sl].reshape(4, 4, LD).transpose(1, 0, 2)
                ),
                "mn": np.ascontiguousarray(
                    mn[sl].reshape(4, 4, LD).transpose(1, 0, 2)
                ),
                "blk4": blk4,
                "e4": e4,
                "ej4": ej4,
                "i4": i4,
            }
        )
    return maps


def run(inputs, **kw):
    """Run on 8 cores; returns (out [128,2] fp32, BassKernelResults)."""
    nc = _get_nc()
    res = run_bass_kernel_spmd(nc, _in_maps(inputs), list(range(NCORES)), **kw)
    out = np.concatenate(
        [np.asarray(res.results[c]["out"], dtype=np.float32) for c in range(NCORES)],
        axis=0,
    )
    return out, res


def kernel(**inputs) -> np.ndarray:
    out, _ = run(inputs)
    return out


# revision 27
# speedup vs baseline: 1.0625x; 1.0625x over previous
"""ColBERT MaxSim retrieval kernel for 8 Trainium2 NeuronCores.

Problem (per reference):
  Q  = l2norm(q_hidden @ W + b)                    [B, 32, 128]
  PD = l2norm((pd_hidden @ W + b) * pd_mask)       [B, 512, 128]
  ND = l2norm((nd_hidden @ W + b) * nd_mask)       [B, 512, 128]
  pos = einsum(Q, PD).max(k).sum(q);  neg likewise; out = [B, 2]

Sharding: pure data parallelism - batch dim (128) split across 8 cores
(16 batches each); W, b replicated.

Key layout decision: the PE contracts along the partition dim, so doc
hidden states must be fed as X^T [H-part, L-free]. Rather than burn PE
cycles transposing on-chip (the v0 bottleneck: 792 PE transposes +
PSUM->SBUF copies), the host pre-transposes each 512-token tile into
[128, 6, 512] bf16 blocks laid out so every per-partition DMA read is
one contiguous 6KB stream.

Normalization trick: never materialize normalized PD. With
  S_raw[q,k] = (Qn @ (Xd W + b)^T)[q,k]
  cs[k] = mask[k] * rsqrt(||Xd_k W + b||^2)
scores are S_raw * cs (masked columns exactly 0, matching the reference
where masked tokens are zero vectors), so pos = sum_q max_k (S_raw*cs).

Per-tile PE work is just: 6-chain projection matmul, one row-sum matmul
(norms, packed 4 tiles/psum-tile via tile_position), one MaxSim matmul.
Per group of 4 tiles: a single K=4 broadcast matmul expands the 4 norm
rows to the 128 score partitions.
"""

import os
import sys

import numpy as np

for _p in ("/opt/trn_rl_repo",):
    if _p not in sys.path and os.path.isdir(_p):
        sys.path.insert(0, _p)

import ml_dtypes  # noqa: E402

import concourse.bass as bass  # noqa: E402
import concourse.bacc as bacc  # noqa: E402
import concourse.tile as tile  # noqa: E402
from concourse import mybir  # noqa: E402
from concourse.bass_utils import run_bass_kernel_spmd  # noqa: E402

# Problem shape (hardcoded per contract)
B, LQ, LD, H, D = 128, 32, 512, 768, 128
NCORES = 8
BC = B // NCORES          # 16 batches per core
KT = H // 128             # 6 contraction chunks

F32 = mybir.dt.float32
BF16 = mybir.dt.bfloat16
AF = mybir.ActivationFunctionType
ALU = mybir.AluOpType

BF16_NP = ml_dtypes.bfloat16


def build_kernel():
    nc = bacc.Bacc()

    # Pre-transposed per-tile layouts: [tile, 128 (h%128), KT*512 (h//128, l)]
    qt_d = nc.dram_tensor("qt", [128, KT * LD], BF16, kind="ExternalInput")
    pdt_d = nc.dram_tensor("pdt", [BC, 128, KT * LD], BF16, kind="ExternalInput")
    ndt_d = nc.dram_tensor("ndt", [BC, 128, KT * LD], BF16, kind="ExternalInput")
    w_d = nc.dram_tensor("W", [128, KT, 128], BF16, kind="ExternalInput")
    b_d = nc.dram_tensor("b", [D, 1], F32, kind="ExternalInput")
    # masks as [j, u, l]: BIG * (1 - mask[4u+j, l]) — added to squared norms
    # so masked tokens get inverse-norm ~1e-9 (scores ~1e-18, below tol)
    mp_d = nc.dram_tensor("mp", [4, 4, LD], BF16, kind="ExternalInput")
    mn_d = nc.dram_tensor("mn", [4, 4, LD], BF16, kind="ExternalInput")
    i4_d = nc.dram_tensor("i4", [4, 4], BF16, kind="ExternalInput")
    blk4_d = nc.dram_tensor("blk4", [4, 128], BF16, kind="ExternalInput")
    e4_d = nc.dram_tensor("e4", [128, 4], BF16, kind="ExternalInput")
    # ej4[p, j, c] = (c == j): routes a full-column reduction to out row j
    ej4_d = nc.dram_tensor("ej4", [128, 4, 4], BF16, kind="ExternalInput")
    out_d = nc.dram_tensor("out", [BC, 2], F32, kind="ExternalOutput")

    with tile.TileContext(nc) as tc:
        with (
            tc.tile_pool(name="const", bufs=1) as const,
            tc.tile_pool(name="xin", bufs=6) as xin,
            tc.tile_pool(name="ptb", bufs=4) as ptbp,
            tc.tile_pool(name="sq", bufs=4) as sqp,
            tc.tile_pool(name="small", bufs=4) as smallp,
            tc.tile_pool(name="csr", bufs=2) as csrp,
            tc.tile_pool(name="persist", bufs=1) as persist,
            tc.tile_pool(name="ptps", bufs=3, space="PSUM") as ptpsp,
            tc.tile_pool(name="ssps", bufs=2, space="PSUM") as sspsp,
            tc.tile_pool(name="s4ps", bufs=2, space="PSUM") as s4psp,
            tc.tile_pool(name="bcps", bufs=1, space="PSUM") as bcpsp,
        ):
            # ---- constants ----
            w_sb = const.tile([128, KT, 128], BF16)
            nc.sync.dma_start(out=w_sb, in_=w_d[:, :, :])
            bias_sb = const.tile([128, 1], F32)
            nc.sync.dma_start(out=bias_sb, in_=b_d[:, :])
            mp_sb = const.tile([4, 4, LD], BF16)
            nc.sync.dma_start(out=mp_sb, in_=mp_d[:, :, :])
            mn_sb = const.tile([4, 4, LD], BF16)
            nc.sync.dma_start(out=mn_sb, in_=mn_d[:, :, :])
            blk4_sb = const.tile([4, 128], BF16)
            nc.sync.dma_start(out=blk4_sb, in_=blk4_d[:, :])
            e4_sb = const.tile([128, 4], BF16)
            nc.sync.dma_start(out=e4_sb, in_=e4_d[:, :])
            ej4_sb = const.tile([128, 4, 4], BF16)
            nc.sync.dma_start(out=ej4_sb, in_=ej4_d[:, :, :])
            i4_sb = const.tile([4, 4], BF16)
            nc.sync.dma_start(out=i4_sb, in_=i4_d[:, :])
            ones_col = const.tile([128, 1], BF16)
            nc.vector.memset(ones_col, 1.0)
            ones_row = const.tile([1, 128], BF16)
            nc.vector.memset(ones_row, 1.0)

            # warm the scalar activation tables while DMAs run
            warm_sb = const.tile([1, 2], BF16)
            nc.scalar.activation(warm_sb, ones_row[0:1, 0:2], AF.Square)
            nc.scalar.activation(warm_sb, ones_row[0:1, 0:2], AF.Abs_reciprocal_sqrt)

            rm_sb = persist.tile([128, 8], BF16)
            qtn_sb = persist.tile([128, BC * LQ], BF16)

            def project(xt_sb):
                """6-chain matmul: xt [128, KT, 512] -> P^T psum [128, 512]."""
                pt_ps = ptpsp.tile([128, LD], F32, tag="pt")
                for k in range(KT):
                    nc.tensor.matmul(
                        pt_ps,
                        w_sb[:, k, :],
                        xt_sb[:, k, :],
                        start=(k == 0),
                        stop=(k == KT - 1),
                    )
                return pt_ps

            # ---- query stage: all 16 batches (512 query tokens) at once ----
            preloaded = {}
            qx_sb = xin.tile([128, KT, LD], BF16, tag="x")
            nc.gpsimd.dma_start(
                out=qx_sb, in_=qt_d[:, :].rearrange("p (k l) -> p k l", k=KT)
            )
            qpt_ps = project(qx_sb)
            qsq_sb = sqp.tile([128, LD], BF16, tag="sq")
            nc.scalar.activation(qsq_sb, qpt_ps, AF.Square, bias=bias_sb)
            qss_ps = sspsp.tile([4, LD], F32, tag="ss")
            nc.tensor.matmul(
                qss_ps[0:1, :], ones_col, qsq_sb, start=True, stop=True
            )
            qinv_sb = smallp.tile([1, LD], BF16, tag="inv")
            nc.scalar.activation(qinv_sb, qss_ps[0:1, :], AF.Abs_reciprocal_sqrt)
            qbc_ps = bcpsp.tile([128, LD], F32, tag="bc")
            nc.tensor.matmul(qbc_ps, ones_row, qinv_sb, start=True, stop=True)
            qtb_sb = ptbp.tile([128, LD], BF16, tag="ptb")
            nc.vector.tensor_scalar_add(qtb_sb, qpt_ps, bias_sb)
            nc.vector.tensor_mul(qtn_sb, qtb_sb, qbc_ps)

            # ---- doc loop: 4 groups x {pd, nd} x 4 tiles ----
            # Group post-processing is deferred ~2 tiles into the next group
            # so its scalar/vector work doesn't contend with the next tiles'
            # bias-add/square at the group seam.
            def emit_post(u, ti, ss_ps, s4_ps, split=False):
                c = 2 * u + ti
                csrm_sb = csrp.tile([4, LD], BF16, tag="csrm")
                nc.scalar.activation(csrm_sb, ss_ps, AF.Abs_reciprocal_sqrt)
                cs_ps = bcpsp.tile([128, LD], F32, tag="bc")
                csb_sb = ptbp.tile([128, LD], BF16, tag="csb")
                scr_sb = sqp.tile([128, LD], BF16, tag="scr")
                halves = 2 if split else 1
                for h in range(halves):
                    pr = slice(128 // halves * h, 128 // halves * (h + 1))
                    rr = slice(4 // halves * h, 4 // halves * (h + 1))
                    nc.tensor.matmul(
                        cs_ps[pr, :],
                        blk4_sb[rr, pr],
                        csrm_sb[rr, :],
                        start=True,
                        stop=True,
                        tile_position=(0, 128 // halves * h),
                    )
                    nc.scalar.copy(csb_sb[pr, :], cs_ps[pr, :])
                    nc.vector.tensor_mul(scr_sb[pr, :], s4_ps[pr, :], csb_sb[pr, :])
                    nc.vector.tensor_reduce(
                        rm_sb[pr, c : c + 1],
                        scr_sb[pr, :],
                        axis=mybir.AxisListType.X,
                        op=ALU.max,
                    )

            groups = [
                (u, ti, xdram, m_sb)
                for u in range(4)
                for ti, (xdram, m_sb) in enumerate(((pdt_d, mp_sb), (ndt_d, mn_sb)))
            ]
            pending = None
            for u, ti, xdram, m_sb in groups:
                ss_ps = sspsp.tile([4, LD], F32, tag="ss")
                s4_ps = s4psp.tile([128, LD], F32, tag="s4")
                for j in range(4):
                    b = 4 * u + j
                    if ti == 0 and b in preloaded:
                        xt_sb = preloaded.pop(b)
                    else:
                        xt_sb = xin.tile([128, KT, LD], BF16, tag="x")
                        nc.gpsimd.dma_start(
                            out=xt_sb,
                            in_=xdram[b, :, :].rearrange("p (k l) -> p k l", k=KT),
                        )
                    pt_ps = project(xt_sb)
                    ptb_sb = ptbp.tile([128, LD], BF16, tag="ptb")
                    nc.vector.tensor_scalar_add(ptb_sb, pt_ps, bias_sb)
                    sq_sb = sqp.tile([128, LD], BF16, tag="sq")
                    nc.scalar.activation(sq_sb, pt_ps, AF.Square, bias=bias_sb)
                    nc.tensor.matmul(
                        ss_ps,
                        ej4_sb[:, j, :],
                        sq_sb,
                        start=(j == 0),
                        stop=False,
                    )
                    nc.tensor.matmul(
                        s4_ps[32 * j : 32 * (j + 1), :],
                        qtn_sb[:, b * LQ : (b + 1) * LQ],
                        ptb_sb,
                        start=True,
                        stop=True,
                        tile_position=(0, 32 * j),
                    )
                    if pending is not None and j == 1:
                        emit_post(*pending)
                        pending = None
                # close the ss chain: +BIG on masked columns
                nc.tensor.matmul(
                    ss_ps, i4_sb, m_sb[:, u, :], start=False, stop=True
                )
                pending = (u, ti, ss_ps, s4_ps)
            emit_post(*pending)

            # ---- final reduction over queries + output ----
            o44_ps = bcpsp.tile([4, 8], F32, tag="bc")
            nc.tensor.matmul(o44_ps, e4_sb, rm_sb, start=True, stop=True)
            o44_sb = smallp.tile([4, 8], F32, tag="o44sb")
            nc.scalar.copy(o44_sb, o44_ps)
            nc.sync.dma_start(
                out=out_d[:, :].rearrange("(u g) t -> g u t", g=4),
                in_=o44_sb.rearrange("g (u t) -> g u t", t=2),
            )

    nc.compile()
    return nc


_NC_CACHE = None


def _get_nc():
    global _NC_CACHE
    if _NC_CACHE is None:
        _NC_CACHE = build_kernel()
    return _NC_CACHE


def _tileize(x):
    """[rows, H] fp32 -> [rows//512, 128, KT*512] bf16, pre-transposed."""
    nt = x.shape[0] // LD
    xb = x.astype(BF16_NP).reshape(nt, LD, KT, 128).transpose(0, 3, 2, 1)
    return np.ascontiguousarray(xb).reshape(nt, 128, KT * LD)


def _in_maps(inputs):
    q = np.asarray(inputs["q_hidden"], dtype=np.float32)
    pd = np.asarray(inputs["pd_hidden"], dtype=np.float32)
    nd = np.asarray(inputs["nd_hidden"], dtype=np.float32)
    W = np.asarray(inputs["W"], dtype=np.float32)
    b = np.ascontiguousarray(
        np.asarray(inputs["b"], dtype=np.float32).reshape(D, 1)
    )
    w_t = np.ascontiguousarray(
        W.astype(BF16_NP).reshape(KT, 128, D).transpose(1, 0, 2)
    )
    # masks [B, LD] -> per-core [4(j), 4(u), LD] bf16 = BIG * (1 - mask)
    MASK_BIG = 1.0e18
    mp = (
        (1.0 - np.asarray(inputs["pd_mask"], dtype=np.float32)) * MASK_BIG
    ).astype(BF16_NP)
    mn = (
        (1.0 - np.asarray(inputs["nd_mask"], dtype=np.float32)) * MASK_BIG
    ).astype(BF16_NP)
    i4 = np.eye(4, dtype=BF16_NP)
    blk4 = np.zeros((4, 128), dtype=BF16_NP)
    for j in range(4):
        blk4[j, 32 * j : 32 * (j + 1)] = 1
    e4 = np.zeros((128, 4), dtype=BF16_NP)
    for g in range(4):
        e4[32 * g : 32 * (g + 1), g] = 1
    ej4 = np.zeros((128, 4, 4), dtype=BF16_NP)
    for j in range(4):
        ej4[:, j, j] = 1
    maps = []
    for c in range(NCORES):
        sl = slice(c * BC, (c + 1) * BC)
        maps.append(
            {
                "qt": _tileize(q[sl].reshape(BC * LQ, H)).reshape(128, KT * LD),
                "pdt": _tileize(pd[sl].reshape(BC * LD, H)),
                "ndt": _tileize(nd[sl].reshape(BC * LD, H)),
                "W": w_t,
                "b": b,
                "mp": np.ascontiguousarray(
                    mp[sl].reshape(4, 4, LD).transpose(1, 0, 2)
                ),
                "mn": np.ascontiguousarray(
                    mn[sl].reshape(4, 4, LD).transpose(1, 0, 2)
                ),
                "blk4": blk4,
                "e4": e4,
                "ej4": ej4,
                "i4": i4,
            }
        )
    return maps


def run(inputs, **kw):
    """Run on 8 cores; returns (out [128,2] fp32, BassKernelResults)."""
    nc = _get_nc()
    res = run_bass_kernel_spmd(nc, _in_maps(inputs), list(range(NCORES)), **kw)
    out = np.concatenate(
        [np.asarray(res.results[c]["out"], dtype=np.float32) for c in range(NCORES)],
        axis=0,
    )
    return out, res


def kernel(**inputs) -> np.ndarray:
    out, _ = run(inputs)
    return out
